# revision 4
# baseline (speedup 1.0000x reference)
"""Causal MHA (B=4, S=2048, D=1024, H=16) on 8 TRN2 cores — v2.

Core c: batch c//2, head-group c%2 (8 of 16 heads). All-bf16 data path
(fp32 PSUM accum), DMA-engine x-transpose, natural-V projection, DVE
reciprocal softmax denominator, GPSIMD causal corner zeroing, and
phase-interleaved emission (QKV half1 inside qb0 attention, proj half0
inside qb1 attention).
"""
import sys

for _p in ("/opt/trn_rl_repo", "/root/.axon_site/_ro/trn_rl_repo"):
    if _p not in sys.path:
        sys.path.append(_p)

import numpy as np
from ml_dtypes import bfloat16

import concourse.bass as bass
import concourse.mybir as mybir
import concourse.tile as tile
from concourse import bacc

B, S, D, H = 4, 2048, 1024, 16
HD = D // H            # 64
NHL = 8                # heads per core
QB = 1024              # attention q-block
NKC = S // 128         # 16 k-chunks
dt = mybir.dt
AF = mybir.ActivationFunctionType
ALU = mybir.AluOpType
P = 128


def build_nc(repeat=1):
    nc = bacc.Bacc("TRN2", target_bir_lowering=False, debug=False)

    xs16 = nc.dram_tensor("xs16", [S, D], dt.bfloat16, kind="ExternalInput")
    wqk = nc.dram_tensor("wqk", [P, 8, 8, P], dt.bfloat16, kind="ExternalInput")
    wv = nc.dram_tensor("wv", [P, 8, 512], dt.bfloat16, kind="ExternalInput")
    wpj = nc.dram_tensor("wpj", [P, 4, D], dt.bfloat16, kind="ExternalInput")
    out = nc.dram_tensor("out", [S, D], dt.bfloat16, kind="ExternalOutput")

    from contextlib import ExitStack
    with tile.TileContext(nc) as tc, ExitStack() as _rep:
        if repeat > 1:
            _rep.enter_context(tc.For_i(0, repeat, 1))
        with tc.tile_pool(name="persist", bufs=1) as pp, \
             tc.tile_pool(name="xt", bufs=2) as xtp, \
             tc.tile_pool(name="st", bufs=4) as stp, \
             tc.tile_pool(name="pt", bufs=4) as ptp, \
             tc.tile_pool(name="rs", bufs=4) as rsp, \
             tc.tile_pool(name="psB", bufs=3, space="PSUM") as psB, \
             tc.tile_pool(name="psV", bufs=1, space="PSUM") as psV:

            QT = pp.tile([P, 4, S], dt.bfloat16, tag="QT")   # [hd-in-pair, pair, s]
            KT = pp.tile([P, 4, S], dt.bfloat16, tag="KT")
            # [k, head, kc, col]: 64 V cols then 32 ones (row-sum trick)
            V2 = pp.tile([P, NHL, NKC, 96], dt.bfloat16, tag="V2")
            yT = pp.tile([P, 4, S], dt.bfloat16, tag="yT")
            nc.gpsimd.memset(V2[:, :, :, 64:96], 1.0)

            wqk16 = pp.tile([P, 8, 8, P], dt.bfloat16, tag="wqk16")
            wv16 = pp.tile([P, 8, 512], dt.bfloat16, tag="wv16")
            wpj_r = pp.tile([P, 4, D], dt.bfloat16, tag="wpj_r")
            xT0 = xtp.tile([P, 8, 1024], dt.bfloat16, tag="xT")
            xT1 = xtp.tile([P, 8, 1024], dt.bfloat16, tag="xT")
            xT = [xT0, xT1]
            nc.sync.dma_start(wqk16[:, :, 0:2, :], wqk[:, :, 0:2, :])
            nc.sync.dma_start_transpose(xT[0][:, :, 0:512], xs16[0:512, :])
            nc.sync.dma_start(wqk16[:, :, 2:8, :], wqk[:, :, 2:8, :])
            nc.sync.dma_start_transpose(xT[0][:, :, 512:1024], xs16[512:1024, :])
            nc.sync.dma_start(wv16[:], wv[:])
            nc.sync.dma_start(wpj_r[:], wpj[:])
            nc.sync.dma_start_transpose(xT[1][:], xs16[1024:2048, :])

            def qk_unit(half, sb, chp):
                """ch pair (2chp, 2chp+1) over s-block sb (512) of half."""
                ps = psB.tile([P, QB], dt.float32, tag="ps")
                for j in range(2):
                    ch = 2 * chp + j
                    for dc in range(8):
                        nc.tensor.matmul(
                            ps[:, j * 512:(j + 1) * 512],
                            wqk16[:, dc, ch, :],
                            xT[half][:, dc, sb * 512:(sb + 1) * 512],
                            start=(dc == 0), stop=(dc == 7))
                soff = half * 1024 + sb * 512
                dst = QT if chp < 2 else KT
                pr0 = (2 * chp) % 4
                nc.vector.tensor_copy(
                    dst[:, pr0:pr0 + 2, soff:soff + 512],
                    ps[:].rearrange("p (j f) -> p j f", j=2))

            def v_unit(half, scp):
                """s-chunk pair (2scp, 2scp+1) within half -> V natural."""
                ps = psB.tile([P, QB], dt.float32, tag="ps")
                for j in range(2):
                    sl = (2 * scp + j) * 128
                    for dc in range(8):
                        nc.tensor.matmul(
                            ps[:, j * 512:(j + 1) * 512],
                            xT[half][:, dc, sl:sl + 128],
                            wv16[:, dc, :],
                            start=(dc == 0), stop=(dc == 7))
                kc0 = half * 8 + 2 * scp
                src = ps[:].rearrange("p (j h f) -> p h j f", j=2, h=8)
                nc.vector.tensor_copy(V2[:, :, kc0:kc0 + 2, 0:64], src)

            pend_out = []

            def flush_out():
                while pend_out:
                    sc, so = pend_out.pop(0)
                    nc.sync.dma_start(out[sc * P:(sc + 1) * P, :], so[:])

            def proj_unit(sc, defer=False):
                ps = psB.tile([P, QB], dt.float32, tag="ps")
                for oc in range(2):
                    for pc in range(4):
                        nc.tensor.matmul(
                            ps[:, oc * 512:(oc + 1) * 512],
                            yT[:, pc, sc * P:(sc + 1) * P],
                            wpj_r[:, pc, oc * 512:(oc + 1) * 512],
                            start=(pc == 0), stop=(pc == 3))
                so = stp.tile([P, QB], dt.bfloat16, tag="so")
                nc.vector.tensor_copy(so[:], ps[:])
                if defer:
                    pend_out.append((sc, so))
                else:
                    nc.sync.dma_start(out[sc * P:(sc + 1) * P, :], so[:])

            def attn(h, qb, inject=None):
                pr, parity = h // 2, h % 2
                half = slice(0, 64) if parity == 0 else slice(64, P)
                nkc = (qb + 1) * 8
                base = qb * QB
                pv = psV.tile([P, QB], dt.float32, tag="pv")

                def pv_out(a, b):
                    return pv[0:96, a:b]

                def emit_pv(kc, pT_t, qlo, poff):
                    segs = []
                    if qlo < 512:
                        segs.append((qlo, 512, kc == lastA))
                    segs.append((max(qlo, 512), QB, kc == lastB))
                    for a, b, sp in segs:
                        nc.tensor.matmul(
                            pv_out(a, b), V2[:, h, kc, :],
                            pT_t[:, poff + a - qlo:poff + b - qlo],
                            start=(kc == 0), stop=sp,
                            skip_group_check=True)

                # chunk groups sharing one PSUM tile + one exp: greedy
                # first-fit bin-packing of chunk widths into 1024-col tiles
                groups = []
                for kc in range(nkc):
                    qlo = max(0, kc * P - base)
                    w = QB - qlo
                    placed = False
                    for g in groups:
                        used = sum(QB - q for _, q, _ in g)
                        if used + w <= QB:
                            g.append((kc, qlo, used))
                            placed = True
                            break
                    if not placed:
                        groups.append([(kc, qlo, 0)])
                order = [kc for g in groups for kc, _, _ in g]
                lastA = [kc for kc in order if max(0, kc * P - base) < 512][-1]
                lastB = order[-1]

                pend = []
                for gi, g in enumerate(groups):
                    ps = psB.tile([P, QB], dt.float32, tag="ps")
                    lo = min(poff for _, _, poff in g)
                    hi = max(poff + QB - qlo for _, qlo, poff in g)
                    for kc, qlo, poff in g:
                        a = poff
                        end = poff + QB - qlo
                        while a < end:
                            b = min((a // 512 + 1) * 512, end)
                            nc.tensor.matmul(
                                ps[:, a:b],
                                KT[half, pr, kc * P:(kc + 1) * P],
                                QT[half, pr, base + qlo + a - poff:
                                   base + qlo + b - poff],
                                start=True, stop=True)
                            a = b
                    pT_t = ptp.tile([P, QB], dt.bfloat16, tag="pT")
                    nc.scalar.activation(pT_t[:, lo:hi], ps[:, lo:hi],
                                         AF.Exp, scale=0.125)
                    for kc, qlo, poff in g:
                        if kc * P >= base:  # diagonal: zero k>q corner
                            nc.gpsimd.affine_select(
                                out=pT_t[:, poff:poff + P],
                                in_=pT_t[:, poff:poff + P],
                                compare_op=ALU.is_ge, fill=0.0,
                                base=0, pattern=[[1, P]], channel_multiplier=-1)
                    for item in pend:
                        emit_pv(*item)
                    pend = [(kc, pT_t, qlo, poff) for kc, qlo, poff in g]
                    if inject is not None and gi in inject:
                        inject[gi]()
                for item in pend:
                    emit_pv(*item)

                # reciprocal of row sums straight from PSUM; evacuate y to
                # SBUF via DVE so the single PSUM accumulator frees quickly;
                # odd heads DMA-shift the bf16 product into partitions 64:128
                pvS = stp.tile([P, QB], dt.float32, tag="pvS")
                rsh = rsp.tile([P, QB], dt.float32, tag="rsh")
                nc.vector.reciprocal(rsh[64:96, :], pv[64:96, :])
                nc.vector.tensor_copy(pvS[0:64, :], pv[0:64, :])
                nc.sync.dma_start(rsh[0:32, :], rsh[64:96, :])
                nc.sync.dma_start(rsh[32:64, :], rsh[64:96, :])
                ysl = slice(qb * QB, qb * QB + QB)
                if parity == 0:
                    nc.vector.tensor_tensor(
                        yT[0:64, pr, ysl], pvS[0:64, :], rsh[0:64, :], ALU.mult)
                else:
                    ytmp = rsp.tile([64, QB], dt.bfloat16, tag="ytmp")
                    nc.vector.tensor_tensor(
                        ytmp[:], pvS[0:64, :], rsh[0:64, :], ALU.mult)
                    nc.sync.dma_start(yT[64:P, pr, ysl], ytmp[:])

            # ---- stage 1: QKV for s-half 0 ----
            for sb in range(2):
                for chp in range(4):
                    qk_unit(0, sb, chp)
            for scp in range(4):
                v_unit(0, scp)

            # ---- stage 2: qb0 attention, half-1 QKV interleaved ----
            h1_units = [lambda sb=sb, chp=chp: qk_unit(1, sb, chp)
                        for sb in range(2) for chp in range(4)]
            h1_units += [lambda scp=scp: v_unit(1, scp) for scp in range(4)]
            for h in range(NHL):
                attn(h, 0)
                h1_units[h]()

            # ---- stage 3: qb1 attention, proj half-0 + V half-1 interleaved ----
            vinj = {1 + 2 * i: h1_units[8 + i] for i in range(4)}
            for h in range(NHL):
                if pend_out:
                    flush_out()   # emit head h-1's out DMA behind h's norm DMAs
                attn(h, 1, inject=vinj if h == 0 else None)
                proj_unit(h, defer=True)

            # ---- stage 4: proj half-1 ----
            flush_out()
            for sc in range(8, 16):
                proj_unit(sc)

    nc.compile()
    return nc


def prepare_inputs(x, Wqkv, Wproj):
    """Per-core inputs. Core c: batch c//2, heads (c%2)*8 .. +8."""
    x = np.asarray(x, dtype=np.float32)
    Wqkv = np.asarray(Wqkv, dtype=np.float32)
    Wproj = np.asarray(Wproj, dtype=np.float32)
    Wq = Wqkv[:, :D].reshape(8, P, H, HD)        # [dc, p, head, hd]
    Wk = Wqkv[:, D:2 * D].reshape(8, P, H, HD)
    Wv_ = Wqkv[:, 2 * D:].reshape(8, P, H, HD)
    in_maps = []
    for c in range(8):
        b, g = c // 2, c % 2
        hg = g * NHL
        wqk = np.empty((P, 8, 8, P), dtype=np.float32)
        for ch in range(4):
            wqk[:, :, ch, 0:64] = Wq[:, :, hg + 2 * ch, :].transpose(1, 0, 2)
            wqk[:, :, ch, 64:P] = Wq[:, :, hg + 2 * ch + 1, :].transpose(1, 0, 2)
            wqk[:, :, ch + 4, 0:64] = Wk[:, :, hg + 2 * ch, :].transpose(1, 0, 2)
            wqk[:, :, ch + 4, 64:P] = Wk[:, :, hg + 2 * ch + 1, :].transpose(1, 0, 2)
        wv = Wv_[:, :, hg:hg + NHL, :].transpose(1, 0, 2, 3).reshape(P, 8, 512)
        wpj = np.empty((P, 4, D), dtype=np.float32)
        for pc in range(4):
            wpj[0:64, pc, :] = Wproj[HD * (hg + 2 * pc):HD * (hg + 2 * pc) + HD, :]
            wpj[64:P, pc, :] = Wproj[HD * (hg + 2 * pc + 1):HD * (hg + 2 * pc + 1) + HD, :]
        in_maps.append({
            "xs16": np.ascontiguousarray(x[b]).astype(bfloat16),
            "wqk": wqk.astype(bfloat16),
            "wv": wv.astype(bfloat16),
            "wpj": wpj.astype(bfloat16),
        })
    return in_maps


def combine_outputs(results):
    out = np.empty((B, S, D), dtype=np.float32)
    for b in range(B):
        out[b] = (results[2 * b]["out"].astype(np.float32)
                  + results[2 * b + 1]["out"].astype(np.float32))
    return out


_NC_CACHE = None


def get_nc():
    global _NC_CACHE
    if _NC_CACHE is None:
        _NC_CACHE = build_nc()
    return _NC_CACHE


def kernel(x, Wqkv, Wproj):
    from concourse.bass_utils import run_bass_kernel_spmd
    nc = get_nc()
    in_maps = prepare_inputs(x, Wqkv, Wproj)
    res = run_bass_kernel_spmd(nc, in_maps, core_ids=list(range(8)))
    return combine_outputs(res.results)


if __name__ == "__main__":
    rng = np.random.default_rng(0)
    x = rng.standard_normal((B, S, D), dtype=np.float32)
    Wqkv = (rng.standard_normal((D, 3 * D), dtype=np.float32) / np.sqrt(D)).astype(np.float32)
    Wproj = (rng.standard_normal((D, D), dtype=np.float32) / np.sqrt(D)).astype(np.float32)
    y = kernel(x, Wqkv, Wproj)
    print("ok", y.shape, float(np.abs(y).max()))
